# revision 21
# baseline (speedup 1.0000x reference)
"""DecodePIF heatmap splatting kernel for Trainium2 (8 NeuronCores, SPMD).

acc[b, y, x] = sum_j conf[b,j] * exp(-((x-mx_j)^2 + (y-my_j)^2) / (2*var_j))
for cells with conf > 0.1.  B=4, grid 68x120 cells, output 4 x 544 x 960 f32.

Strategy
--------
The per-batch accumulator is a separable-Gaussian GEMM:
    acc[b] = gy[b].T @ gx[b],  gy [J, Hf], gx [J, Wf], J = 8160 cells.
Each core owns one (batch, x-half) slab [544, 480] of the output.  The
wall-time budget on the axon-tunneled PJRT path is ~32 ms fixed dispatch
+ ~16 ms per transferred MB, so the kernel ships the minimum viable input
and computes everything else ON DEVICE:

- Host ships one packed tensor per core ([128, 4*34] int16, ~35 KB): per
  cell mx/my in 1/32-px fixed point (i16-exact) and -1/(2*var) / masked
  ln(conf) as f16 bit patterns.  Only cells that are alive (conf > 0.1)
  and whose Gaussian support reaches the core's x-half are shipped
  (~3.9k of 8160; capacity 34*128 = 4352 with farthest-reach drop on
  overflow); dead padding uses lnc = -30000 so exp underflows to exact 0.
- Device, per 128-cell chunk: d = (iota32 - m_q)/32 (one 2-op DVE
  tensor_scalar), s = d*d (DVE), then ONE ScalarE activation
  exp(s * (-1/2v) + lnc) with per-partition scale/bias produces the f16
  Gaussian row; 8 f16 matmuls [K=128, M=120, N=272] accumulate gx^T @ gy
  into 8 PSUM banks across all 34 chunks (start on chunk 0, stop on the
  last).  f32->f16 copy-out + a single contiguous DMA per core.

All 8 cores run the same instruction stream (SPMD); per-core differences
live entirely in the packed input (x-half offset is baked into mx_q).
The schedule is shape-static: no data-dependent chunking, one NEFF for
all inputs; overflow only affects which cells occupy the fixed slots.
"""

import os
import sys

for _p in ("/opt/trn_rl_repo",):
    if os.path.isdir(_p) and _p not in sys.path:
        sys.path.insert(0, _p)

import numpy as np

# ---------------------------------------------------------------- constants
STRIDE = 8
B, CH, CW = 4, 68, 120          # batch, cell-grid height/width
HF, WF = CH * STRIDE, CW * STRIDE  # 544 x 960 output grid
J = CH * CW                     # 8160 cells per batch
MIN_CONF = 0.1
N_CORES = 8

P = 128                         # cells per chunk (PE contraction dim)
NCH = 32                        # chunk capacity per core (see _pack_inputs)
CAP = NCH * P                   # 4352 cells
XH = WF // 2                    # 480: x-half owned by a core
# GEMM tiling: lhsT = gy y-blocks (M <= 128), rhs = gx full width
# (N = 480 f32 = 1920 B, fits one PSUM bank) -> 5 matmuls per chunk.
MTILES = [112, 112, 112, 112, 96]   # y M-tile heights, sum = 544
MOFF = [0, 112, 224, 336, 448]
MP = 112                        # osb partition rows (max tile height)
DEAD_LNC = -30000.0             # dead-cell ln(conf) -> exp == 0
NG = 4                          # packed input groups per cell
T_CUT = 10.0                    # support cutoff: drop cells with no reach
QS = 32.0                       # px fixed-point scale (1/32 px, i16-exact)

_f16 = np.float16
_f32 = np.float32


# ---------------------------------------------------------------- host side
def _pack_inputs(mean, variance, confidence):
    """Per-core packed [128, NG*NCH] int16 tensors.

    Groups: mx_q (i16, (mx - x0) * 32), my_q (i16, my * 32), -1/(2*var)
    (f16 bits), masked ln(conf) (f16 bits).  Only cells that are alive
    (conf > 0.1) AND whose Gaussian support [mx - r, mx + r]
    (r = sqrt(2*var*T_CUT)) intersects the core's x-half are shipped --
    everything else contributes exactly 0 to this core's slab.  Capacity is
    CAP cells; on (pathological) overflow the cells reaching least far into
    the window are dropped.  Dead padding uses lnc = -30000 => gx == 0.
    """
    mx = mean[..., 0].reshape(B, J).astype(np.float64)
    my = mean[..., 1].reshape(B, J).astype(np.float64)
    var = variance.reshape(B, J).astype(np.float64)
    conf = confidence.reshape(B, J).astype(np.float64)

    nega_all = (-1.0 / (2.0 * var)).astype(_f16).view(np.int16)
    lnc_all = np.log(np.maximum(conf, 1e-30)).astype(_f16).view(np.int16)
    dead_lnc = np.array(DEAD_LNC, _f16).view(np.int16)
    dead_nega = np.array(-1.0, _f16).view(np.int16)

    packed = np.zeros((N_CORES, P, NG * NCH), dtype=np.int16)
    for core in range(N_CORES):
        b, xh = core // 2, core % 2
        x0 = XH * xh
        r = np.sqrt(2.0 * var[b] * T_CUT)
        keep = (conf[b] > MIN_CONF) & (mx[b] > x0 - r) & (mx[b] < x0 + XH + r)
        idx = np.nonzero(keep)[0]
        if idx.size > CAP:
            # farthest-outside-the-window cells lose their slot
            reach = (np.abs(mx[b][idx] - (x0 + XH / 2)) - XH / 2) / r[idx]
            idx = idx[np.argpartition(reach, CAP)[:CAP]]
        n = idx.size
        arr = np.empty((NG, CAP), dtype=np.int16)
        arr[0, :n] = np.round((mx[b][idx] - x0) * QS).astype(np.int16)
        arr[1, :n] = np.round(my[b][idx] * QS).astype(np.int16)
        arr[2, :n] = nega_all[b][idx]
        arr[3, :n] = lnc_all[b][idx]
        arr[0, n:] = 0
        arr[1, n:] = 0
        arr[2, n:] = dead_nega
        arr[3, n:] = dead_lnc
        # cell slot = chunk*128 + partition; column = group*NCH + chunk
        packed[core] = arr.reshape(NG, NCH, P).transpose(2, 0, 1).reshape(
            P, NG * NCH)
    return packed


# -------------------------------------------------------------- device side
def _build_nc():
    import concourse.tile as tile
    from concourse import bacc, mybir
    from contextlib import ExitStack

    f16, f32 = mybir.dt.float16, mybir.dt.float32
    i16, i32 = mybir.dt.int16, mybir.dt.int32

    nc = bacc.Bacc("TRN2", target_bir_lowering=False, debug=False,
                   num_devices=N_CORES)
    inp_d = nc.dram_tensor("inp", [P, NG * NCH], i16,
                           kind="ExternalInput").ap()
    out_d = nc.dram_tensor("out", [MP, len(MTILES) * XH], f16,
                           kind="ExternalOutput").ap()

    with tile.TileContext(nc) as tc, ExitStack() as ctx:
        constp = ctx.enter_context(tc.tile_pool(name="const", bufs=1))
        gp = ctx.enter_context(tc.tile_pool(name="g", bufs=3))
        accp = ctx.enter_context(tc.tile_pool(name="acc", bufs=1,
                                              space="PSUM"))
        osbp = ctx.enter_context(tc.tile_pool(name="osb", bufs=1))

        inp = constp.tile([P, NG * NCH], i16)
        nc.sync.dma_start(inp[:], inp_d)

        # iota in 1/QS px units so quantized coords subtract exactly
        iota_i = constp.tile([P, HF], i32)
        nc.gpsimd.iota(iota_i[:], pattern=[[int(QS), HF]], base=0,
                       channel_multiplier=0)
        iota_f = constp.tile([P, HF], f32)
        nc.vector.tensor_copy(iota_f[:], iota_i[:])

        def grp(g):
            return inp[:, g * NCH:(g + 1) * NCH]

        mxf = constp.tile([P, NCH], f32)
        nc.vector.tensor_copy(mxf[:], grp(0))
        myf = constp.tile([P, NCH], f32)
        nc.vector.tensor_copy(myf[:], grp(1))
        negaf = constp.tile([P, NCH], f32)
        nc.scalar.copy(negaf[:], grp(2).bitcast(f16))
        lncf = constp.tile([P, NCH], f32)
        nc.scalar.copy(lncf[:], grp(3).bitcast(f16))

        accs = [accp.tile([MTILES[t], XH], f32, name=f"acc{t}",
                          tag=f"acc{t}")
                for t in range(len(MTILES))]

        inv_qs = float(1.0 / QS)
        for c in range(NCH):
            # x and y distance rows share one tile so a single DVE square
            # covers both
            d = gp.tile([P, XH + HF], f32, name="d", tag="d")
            nc.vector.tensor_scalar(d[:, :XH], iota_f[:, :XH],
                                    mxf[:, c:c + 1], inv_qs,
                                    mybir.AluOpType.subtract,
                                    mybir.AluOpType.mult)
            nc.vector.tensor_scalar(d[:, XH:], iota_f[:], myf[:, c:c + 1],
                                    inv_qs, mybir.AluOpType.subtract,
                                    mybir.AluOpType.mult)
            s = gp.tile([P, XH + HF], f32, name="s", tag="s")
            nc.vector.tensor_tensor(s[:], d[:], d[:], mybir.AluOpType.mult)
            gx = gp.tile([P, XH], f16, name="gx", tag="gx")
            nc.scalar.activation(gx[:], s[:, :XH],
                                 mybir.ActivationFunctionType.Exp,
                                 bias=lncf[:, c:c + 1],
                                 scale=negaf[:, c:c + 1])
            gy = gp.tile([P, HF], f16, name="gy", tag="gy")
            nc.scalar.activation(gy[:], s[:, XH:],
                                 mybir.ActivationFunctionType.Exp,
                                 scale=negaf[:, c:c + 1])
            for t, (yo, mw) in enumerate(zip(MOFF, MTILES)):
                nc.tensor.matmul(
                    accs[t][:],
                    lhsT=gy[:, yo:yo + mw],
                    rhs=gx[:],
                    start=(c == 0), stop=(c == NCH - 1),
                    skip_group_check=True,
                )

        osb = osbp.tile([MP, len(MTILES) * XH], f16)
        for t, mw in enumerate(MTILES):
            nc.vector.tensor_copy(osb[:mw, t * XH:(t + 1) * XH], accs[t][:])
        nc.sync.dma_start(out_d, osb[:])

    nc.compile()
    return nc


# ------------------------------------------------------------------ runner
class _PjrtRunner:
    """Cached jitted SPMD executable; device-resident output placeholders."""

    def __init__(self, nc):
        import jax
        from jax.sharding import Mesh, PartitionSpec
        from jax.experimental.shard_map import shard_map
        from concourse import mybir
        from concourse.bass2jax import (
            _bass_exec_p,
            install_neuronx_cc_hook,
            partition_id_tensor,
        )

        install_neuronx_cc_hook()
        assert nc.dbg_addr is None
        partition_name = (
            nc.partition_id_tensor.name if nc.partition_id_tensor else None
        )
        in_names, out_names, out_avals = [], [], []
        for alloc in nc.m.functions[0].allocations:
            if not isinstance(alloc, mybir.MemoryLocationSet):
                continue
            name = alloc.memorylocations[0].name
            if alloc.kind == "ExternalInput":
                if name != partition_name:
                    in_names.append(name)
            elif alloc.kind == "ExternalOutput":
                shape = tuple(alloc.tensor_shape)
                dtype = mybir.dt.np(alloc.dtype)
                out_names.append(name)
                out_avals.append(jax.core.ShapedArray(shape, dtype))
        all_in_names = list(in_names) + list(out_names)
        if partition_name is not None:
            all_in_names.append(partition_name)

        def _body(*args):
            operands = list(args)
            if partition_name is not None:
                operands.append(partition_id_tensor())
            outs = _bass_exec_p.bind(
                *operands,
                out_avals=tuple(out_avals),
                in_names=tuple(all_in_names),
                out_names=tuple(out_names),
                lowering_input_output_aliases=(),
                sim_require_finite=True,
                sim_require_nnan=True,
                nc=nc,
            )
            return tuple(outs)

        devices = jax.devices()[:N_CORES]
        mesh = Mesh(np.asarray(devices), ("core",))
        n_params = len(in_names)
        n_outs = len(out_avals)
        self._fn = jax.jit(
            shard_map(
                _body, mesh=mesh,
                in_specs=(PartitionSpec("core"),) * (n_params + n_outs),
                out_specs=(PartitionSpec("core"),) * n_outs,
                check_rep=False,
            ),
            keep_unused=True,
        )
        self._in_names = in_names
        self._out_names = out_names
        self._out_avals = out_avals
        # Placeholder buffers for the NEFF ExternalOutput slots.  The
        # compile hook requires them as plain jit parameters, but the NEFF
        # binds its outputs to the custom-call RESULTS (out_rename wins),
        # so these are never read: keep one device-resident copy and reuse
        # it every call -- no per-call host->device traffic, no donation.
        from jax.sharding import NamedSharding
        sh = NamedSharding(mesh, PartitionSpec("core"))
        self._zeros_dev = [
            jax.device_put(
                np.zeros((N_CORES * a.shape[0], *a.shape[1:]), a.dtype), sh)
            for a in out_avals
        ]
        jax.block_until_ready(self._zeros_dev)

    def concat_inputs(self, in_maps):
        return [
            np.concatenate([np.asarray(m[name]) for m in in_maps], axis=0)
            for name in self._in_names
        ]

    def run_raw(self, args):
        return self._fn(*args, *self._zeros_dev)

    def __call__(self, in_maps):
        out_arrs = self.run_raw(self.concat_inputs(in_maps))
        return [
            {
                name: np.asarray(out_arrs[i]).reshape(
                    N_CORES, *self._out_avals[i].shape
                )[c]
                for i, name in enumerate(self._out_names)
            }
            for c in range(N_CORES)
        ]


_CACHE = {}


def _get_runner():
    if "r" not in _CACHE:
        nc = _build_nc()
        _CACHE["r"] = (nc, _PjrtRunner(nc))
    return _CACHE["r"]


def _assemble(results):
    full = np.zeros((B, HF, WF), dtype=_f32)
    for core in range(N_CORES):
        b, xh = core // 2, core % 2
        o = results[core]["out"]            # [MP, len(MTILES)*XH] f16 [y, x]
        x0 = XH * xh
        for t, (yo, mw) in enumerate(zip(MOFF, MTILES)):
            full[b, yo:yo + mw, x0:x0 + XH] = (
                o[:mw, t * XH:(t + 1) * XH].astype(_f32))
    return full


def kernel(mean, variance, confidence):
    mean = np.asarray(mean)
    variance = np.asarray(variance)
    confidence = np.asarray(confidence)
    packed = _pack_inputs(mean, variance, confidence)
    _nc, runner = _get_runner()
    in_maps = [{"inp": packed[c]} for c in range(N_CORES)]
    results = runner(in_maps)
    return _assemble(results)


if __name__ == "__main__":
    rng = np.random.default_rng(0)
    mean = np.stack(
        [
            rng.uniform(0, WF, (B, CH, CW)).astype(_f32),
            rng.uniform(0, HF, (B, CH, CW)).astype(_f32),
        ],
        axis=-1,
    )
    variance = rng.uniform(4.0, 64.0, (B, CH, CW)).astype(_f32)
    confidence = rng.uniform(0, 1, (B, CH, CW)).astype(_f32)
    out = kernel(mean=mean, variance=variance, confidence=confidence)
    print("out", out.shape, out.dtype, out.mean())
